# revision 5
# baseline (speedup 1.0000x reference)
"""Multi-head attention (B=8, S=2048, D=512, H=8) on 8 Trainium2 NeuronCores.

Strategy: pure data parallelism — one batch element per core, no collectives.

Per-core device pipeline (all matmuls fp16 with fp32 PSUM accumulation):
  1. Projections: qT/kT in transposed layout [e, s] (attention contracts
     dk on partitions), v in natural [s, e] layout augmented with a ones
     column per head (the PV matmul then also produces softmax denominators).
     Inputs arrive pre-transposed from host as X^T [c, s] fp16.
  2. Attention per (head-pair, s-half): the two heads of an e-tile live on
     partitions 0-63 / 64-127, so their K=64 score matmuls are issued
     back-to-back as PE row-tiles (tile_position (0,0)/(64,0)) and run
     concurrently. Scores for the pair land in one [128, 2, 1024] PSUM
     tile, exp'd by a single FD=2048 ScalarE activation (PSUM->SBUF fp16),
     masked on DVE (0/1 multiplicative, fp16 2x), then PV matmuls with
     [V|1] stationary accumulate outT rows + denominators in PSUM.
  3. Denominator rows DMA'd straight out of PSUM, DVE reciprocal,
     partition-broadcast via a DRAM bounce, normalize, final projection
     with Wo.T, bias, DMA out.

The final projection is folded into the attention phase: each s-half is
normalized and projected as soon as its denominators are complete, so the
tail after the last attention block is short.  The run is ScalarE-bound
(33.5M exps/core at 1 elem/lane/cycle); PE has ~2x slack.

Softmax note: reference softmax(where(mask==0, -1e30, s)) == exp(s)*mask
normalized — scores are O(1) so no max-subtraction is needed, and the 0/1
mask is exact in fp16. Scale 1/sqrt(dk)=1/8 is folded into Wq/bq on host.
"""
import numpy as np

import concourse.bacc as bacc
import concourse.bass as bass
import concourse.mybir as mybir
import concourse.tile as tile
from concourse.bass_utils import run_bass_kernel_spmd

B, S, D, H, DK = 8, 2048, 512, 8, 64
P = 128            # partition tile
NET = D // P       # 4 e-tiles (contraction chunks / head pairs)
NST = S // P       # 16 s-tiles / j-tiles
SCW = 512          # matmul moving free dim
NSC = S // SCW     # 4
SHW = 1024         # attention s-block width
NSH = S // SHW     # 2

f32 = mybir.dt.float32
fp16 = mybir.dt.float16

_CACHE: dict = {}


def _build():
    nc = bacc.Bacc("TRN2", target_bir_lowering=False, debug=False)

    d_xq = nc.dram_tensor("xq", [D, S], fp16, kind="ExternalInput")
    d_xk = nc.dram_tensor("xk", [D, S], fp16, kind="ExternalInput")
    d_xv = nc.dram_tensor("xv", [D, S], fp16, kind="ExternalInput")
    d_mskT = nc.dram_tensor("mskT", [S, S], fp16, kind="ExternalInput")
    d_wq = nc.dram_tensor("wq", [D, D], fp16, kind="ExternalInput")  # Wq.T/8
    d_wk = nc.dram_tensor("wk", [D, D], fp16, kind="ExternalInput")  # Wk.T
    d_wv = nc.dram_tensor("wv", [D, D], fp16, kind="ExternalInput")  # Wv.T
    d_wo = nc.dram_tensor("wo", [D, D], fp16, kind="ExternalInput")  # Wo.T
    d_bq = nc.dram_tensor("bq", [D], f32, kind="ExternalInput")      # bq/8
    d_bk = nc.dram_tensor("bk", [D], f32, kind="ExternalInput")
    d_bv = nc.dram_tensor("bv", [D], f32, kind="ExternalInput")
    d_bo = nc.dram_tensor("bo", [D], f32, kind="ExternalInput")
    d_out = nc.dram_tensor("out", [S, D], f32, kind="ExternalOutput")
    d_rec = nc.dram_tensor("rec_dram", [H, S], f32)

    Exp = mybir.ActivationFunctionType.Exp

    with tile.TileContext(nc) as tc, \
         tc.tile_pool(name="persist", bufs=1) as persist:

        qT = persist.tile([P, NET, S], fp16)             # [e%128, et, s]
        kT = persist.tile([P, NET, S], fp16)
        v_aug = persist.tile([P, NST, H, DK + 1], fp16)  # [j%128, jt, h, d|1]
        outT = persist.tile([P, NET, S], fp16)           # [hd%128, et, s] unnorm
        denom = persist.tile([P, NSH, 64], f32)
        bq_sb = persist.tile([P, NET], f32)
        bk_sb = persist.tile([P, NET], f32)
        bv_bc = persist.tile([P, D], f32)
        wo_sb = persist.tile([P, NET, D], fp16)
        bo_bc = persist.tile([P, D], f32)
        outTn = persist.tile([P, NET, S], fp16)

        nc.sync.dma_start(out=bq_sb, in_=d_bq.ap().rearrange("(cc p) -> p cc", p=P))
        nc.sync.dma_start(out=bk_sb, in_=d_bk.ap().rearrange("(cc p) -> p cc", p=P))
        nc.sync.dma_start(
            out=bv_bc,
            in_=bass.AP(tensor=d_bv.ap().tensor, offset=0, ap=[[0, P], [1, D]]))
        nc.vector.memset(v_aug[:, :, :, DK:DK + 1], 1.0)

        with tc.tile_pool(name="maskp", bufs=1) as maskp:
          maskT = maskp.tile([P, NST, S], fp16)
          msk_ap = d_mskT.ap().rearrange("(jt p) s -> p jt s", p=P)

          # ---------------- projections (q, k, v) ----------------
          with tc.tile_pool(name="projx", bufs=2) as projx, \
               tc.tile_pool(name="projw", bufs=2) as projw, \
               tc.tile_pool(name="projps", bufs=4, space="PSUM") as projps:
            mask_sched = {0: range(0, 4), 1: range(4, 8), 2: range(8, NST)}
            for which, (d_x, d_w) in enumerate(
                    [(d_xq, d_wq), (d_xk, d_wk), (d_xv, d_wv)]):
                w_sb = projw.tile([P, NET, D], fp16, tag="w", name="w_sb")
                nc.sync.dma_start(
                    out=w_sb, in_=d_w.ap().rearrange("(cc p) e -> p cc e", p=P))
                x_sb = projx.tile([P, NET, S], fp16, tag="x", name="x_sb")
                x_ap = d_x.ap().rearrange("(cc p) s -> p cc s", p=P)
                for cc in range(NET):
                    nc.sync.dma_start(out=x_sb[:, cc, :], in_=x_ap[:, cc, :])
                if which == 0:
                    nc.sync.dma_start(
                        out=wo_sb,
                        in_=d_wo.ap().rearrange("(cc p) e -> p cc e", p=P))
                    nc.sync.dma_start(
                        out=bo_bc,
                        in_=bass.AP(tensor=d_bo.ap().tensor, offset=0,
                                    ap=[[0, P], [1, D]]))
                for jt in mask_sched[which]:
                    nc.sync.dma_start(out=maskT[:, jt, :], in_=msk_ap[:, jt, :])

                if which == 2:  # v -> natural layout [s, e] into v_aug
                    for st in range(NST):
                        ps_t = projps.tile([P, SCW], f32, tag="ps",
                                           name="ps_t")
                        for cc in range(NET):
                            nc.tensor.matmul(
                                ps_t,
                                x_sb[:, cc, st * P:(st + 1) * P],
                                w_sb[:, cc, :],
                                start=(cc == 0), stop=(cc == NET - 1))
                        nc.vector.tensor_add(
                            v_aug[:, st, :, 0:DK],
                            ps_t.rearrange("p (h d) -> p h d", h=H),
                            bv_bc.rearrange("p (h d) -> p h d", h=H))
                else:  # q, k -> transposed layout [e, s]
                    dst = qT if which == 0 else kT
                    bias = bq_sb if which == 0 else bk_sb
                    for et in range(NET):
                        for sc in range(NSC):
                            ps_t = projps.tile([P, SCW], f32, tag="ps",
                                               name="ps_t")
                            for cc in range(NET):
                                nc.tensor.matmul(
                                    ps_t,
                                    w_sb[:, cc, et * P:(et + 1) * P],
                                    x_sb[:, cc, sc * SCW:(sc + 1) * SCW],
                                    start=(cc == 0), stop=(cc == NET - 1))
                            nc.scalar.activation(
                                dst[:, et, sc * SCW:(sc + 1) * SCW], ps_t,
                                mybir.ActivationFunctionType.Identity,
                                bias=bias[:, et:et + 1])

          # ---------------- attention ----------------
          # sh outer: when the first s-half of all head-pairs is done, its
          # final projection runs while the second half computes.
          with tc.tile_pool(name="attn", bufs=4) as attn, \
               tc.tile_pool(name="attnps", bufs=1, space="PSUM") as attnps:
            for sh in range(NSH):
                c0 = sh * SHW
                for et in range(NET):          # head pair (2et, 2et+1)
                    # 4 PV accumulators: (head-in-pair hh, s-chunk i)
                    pvs = [attnps.tile([65, SCW], f32, tag="pv", bufs=4,
                                       name=f"pv{m}") for m in range(4)]
                    for jt in range(NST):
                        # both heads' scores in one 4-bank tile
                        sc_ps = attnps.tile([P, 2, SHW], f32, tag="sc",
                                            bufs=1, name="sc_ps")
                        for i in range(2):
                            for hh in range(2):   # row-tiled pair, concurrent
                                nc.tensor.matmul(
                                    sc_ps[:, hh, i * SCW:(i + 1) * SCW],
                                    kT[hh * DK:(hh + 1) * DK, et,
                                       jt * P:(jt + 1) * P],
                                    qT[hh * DK:(hh + 1) * DK, et,
                                       c0 + i * SCW:c0 + (i + 1) * SCW],
                                    start=True, stop=True)
                        ex = attn.tile([P, 2, SHW], fp16, tag="ex", bufs=3,
                                       name="ex")
                        # per-head exps: subtile deps let next jt's scores
                        # for head hh start as soon as exp(jt, hh) drains
                        for hh in range(2):
                            nc.scalar.activation(ex[:, hh, :],
                                                 sc_ps[:, hh, :], Exp)
                        pb = attn.tile([P, 2, SHW], fp16, tag="pb", bufs=4,
                                       name="pb")
                        msk = maskT[:, jt, c0:c0 + SHW]
                        msk2 = bass.AP(tensor=msk.tensor, offset=msk.offset,
                                       ap=[msk.ap[0], [0, 2]] + msk.ap[1:])
                        nc.vector.tensor_mul(pb, ex, msk2)
                        for hh in range(2):
                            for i in range(2):
                                nc.tensor.matmul(
                                    pvs[hh * 2 + i],
                                    v_aug[:, jt, 2 * et + hh, :],
                                    pb[:, hh, i * SCW:(i + 1) * SCW],
                                    start=(jt == 0), stop=(jt == NST - 1))
                    for hh in range(2):
                        ro = hh * DK
                        for i in range(2):
                            cols = c0 + i * SCW
                            pv_t = pvs[hh * 2 + i]
                            nc.vector.tensor_copy(
                                outT[ro:ro + DK, et, cols:cols + SCW],
                                pv_t[0:DK, :])
                            dst_t = attn.tile([65, SCW], f32, tag="dst",
                                              bufs=2, name="dst_t")
                            nc.vector.tensor_copy(dst_t[64:65, :],
                                                  pv_t[64:65, :])
                            pbase = (2 * et + hh) * 16 + i * 8
                            nc.gpsimd.dma_start(
                                out=denom[pbase:pbase + 8, sh, :],
                                in_=dst_t[64:65, :])
                    # pair done for this half: normalize now
                    rec = attn.tile([32, 64], f32, tag="rec", bufs=2,
                                    name="rec")
                    nc.vector.reciprocal(
                        rec, denom[et * 32:(et + 1) * 32, sh, :])
                    nc.sync.dma_start(
                        out=d_rec.ap()[2 * et:2 * et + 2, c0:c0 + SHW],
                        in_=rec)
                    rb = attn.tile([P, SHW], f32, tag="rb", bufs=2,
                                   name="rb")
                    nc.gpsimd.dma_start(
                        out=rb[0:64, :],
                        in_=bass.AP(tensor=d_rec.ap().tensor,
                                    offset=(2 * et) * S + c0,
                                    ap=[[0, 64], [1, SHW]]))
                    nc.gpsimd.dma_start(
                        out=rb[64:128, :],
                        in_=bass.AP(tensor=d_rec.ap().tensor,
                                    offset=(2 * et + 1) * S + c0,
                                    ap=[[0, 64], [1, SHW]]))
                    nc.vector.tensor_mul(outTn[:, et, c0:c0 + SHW],
                                         outT[:, et, c0:c0 + SHW], rb)
                # final projection for this s-half (PSUM slots shared w/ pv)
                for st in range(sh * NST // NSH, (sh + 1) * NST // NSH):
                    ps_f = attnps.tile([P, D], f32, tag="pv", bufs=4,
                                       name="ps_f")
                    for cc in range(NET):
                        nc.tensor.matmul(
                            ps_f,
                            outTn[:, cc, st * P:(st + 1) * P],
                            wo_sb[:, cc, :],
                            start=(cc == 0), stop=(cc == NET - 1))
                    o_sb = attn.tile([P, D], f32, tag="os", bufs=2,
                                     name="o_sb")
                    nc.vector.tensor_add(o_sb, ps_f, bo_bc)
                    nc.sync.dma_start(
                        out=d_out.ap()[st * P:(st + 1) * P, :], in_=o_sb)

    nc.compile()
    return nc


def _get_nc():
    if "nc" not in _CACHE:
        _CACHE["nc"] = _build()
    return _CACHE["nc"]


def _preprocess(Q, K, V, mask, Wq, bq, Wk, bk, Wv, bv, Wo, bo):
    """Host-side sharding + layout marshalling (per-core input dicts)."""
    mT = np.ascontiguousarray(np.asarray(mask)[0, 0].T).astype(np.float16)
    wq_h = np.ascontiguousarray(np.asarray(Wq).T / 8.0).astype(np.float16)
    wk_h = np.ascontiguousarray(np.asarray(Wk).T).astype(np.float16)
    wv_h = np.ascontiguousarray(np.asarray(Wv).T).astype(np.float16)
    wo_h = np.ascontiguousarray(np.asarray(Wo).T).astype(np.float16)
    bq_h = np.asarray(bq, dtype=np.float32) / 8.0
    bk_h = np.asarray(bk, dtype=np.float32)
    bv_h = np.asarray(bv, dtype=np.float32)
    bo_h = np.asarray(bo, dtype=np.float32)
    Q, K, V = np.asarray(Q), np.asarray(K), np.asarray(V)
    in_maps = []
    for b in range(B):
        in_maps.append({
            "xq": np.ascontiguousarray(Q[b].T).astype(np.float16),
            "xk": np.ascontiguousarray(K[b].T).astype(np.float16),
            "xv": np.ascontiguousarray(V[b].T).astype(np.float16),
            "mskT": mT,
            "wq": wq_h, "wk": wk_h, "wv": wv_h, "wo": wo_h,
            "bq": bq_h, "bk": bk_h, "bv": bv_h, "bo": bo_h,
        })
    return in_maps


def run(inputs: dict, trace: bool = False):
    nc = _get_nc()
    in_maps = _preprocess(**inputs)
    res = run_bass_kernel_spmd(nc, in_maps, core_ids=list(range(B)), trace=trace)
    outp = np.stack([res.results[b]["out"] for b in range(B)], axis=0)
    return outp.astype(np.float32), res


def kernel(**inputs) -> np.ndarray:
    outp, _ = run(inputs, trace=False)
    return outp


# revision 8
# speedup vs baseline: 1.1924x; 1.1924x over previous
"""Multi-head attention (B=8, S=2048, D=512, H=8) on 8 Trainium2 NeuronCores.

Strategy: pure data parallelism — one batch element per core, no collectives.

Per-core device pipeline (all matmuls fp16 with fp32 PSUM accumulation):
  1. Projections: qT/kT in transposed layout [e, s] (attention contracts
     dk on partitions), v in natural [s, e] layout augmented with a ones
     column per head (the PV matmul then also produces softmax denominators).
     Inputs arrive pre-transposed from host as X^T [c, s] fp16.
  2. Attention per (head-pair, s-block of 512): the two heads of an e-tile
     live on partitions 0-63 / 64-127, so their K=64 score matmuls are
     issued back-to-back as PE row-tiles (tile_position (0,0)/(64,0)) and
     run concurrently into one [128, 2, 512] PSUM tile (2 banks,
     double-buffered).  One FD=1024 ScalarE exp covers the pair
     (PSUM->SBUF fp16), one DVE multiply applies the 0/1 mask to both
     heads via a zero-stride broadcast AP, then two PV matmuls with [V|1]
     stationary accumulate outT rows + denominators in PSUM.  The
     pipeline is ScalarE-bound (33.5M exps/core at 1 elem/lane/cycle);
     PE and DVE have slack.
  3. Denominator rows staged to SBUF, DVE reciprocal, partition-broadcast
     via a DRAM bounce, normalize, final projection with Wo.T, bias, DMA
     out — all per s-block, so the tail after the last attention block is
     short.

Softmax note: reference softmax(where(mask==0, -1e30, s)) == exp(s)*mask
normalized — scores are O(1) so no max-subtraction is needed, and the 0/1
mask is exact in fp16. Scale 1/sqrt(dk)=1/8 is folded into Wq/bq on host.
"""
import numpy as np

import concourse.bacc as bacc
import concourse.bass as bass
import concourse.mybir as mybir
import concourse.tile as tile
from concourse.bass_utils import run_bass_kernel_spmd

B, S, D, H, DK = 8, 2048, 512, 8, 64
P = 128            # partition tile
NET = D // P       # 4 e-tiles (contraction chunks / head pairs)
NST = S // P       # 16 s-tiles / j-tiles
SCW = 512          # matmul moving free dim
NSC = S // SCW     # 4
SHW = 512          # attention s-block width
NSH = S // SHW     # 4

f32 = mybir.dt.float32
fp16 = mybir.dt.float16

_CACHE: dict = {}


def _build():
    nc = bacc.Bacc("TRN2", target_bir_lowering=False, debug=False)

    d_xq = nc.dram_tensor("xq", [D, S], fp16, kind="ExternalInput")
    d_xk = nc.dram_tensor("xk", [D, S], fp16, kind="ExternalInput")
    d_xv = nc.dram_tensor("xv", [D, S], fp16, kind="ExternalInput")
    d_mskT = nc.dram_tensor("mskT", [S, S], fp16, kind="ExternalInput")
    d_wq = nc.dram_tensor("wq", [D, D], fp16, kind="ExternalInput")  # Wq.T/8
    d_wk = nc.dram_tensor("wk", [D, D], fp16, kind="ExternalInput")  # Wk.T
    d_wv = nc.dram_tensor("wv", [D, D], fp16, kind="ExternalInput")  # Wv.T
    d_wo = nc.dram_tensor("wo", [D, D], fp16, kind="ExternalInput")  # Wo.T
    d_bq = nc.dram_tensor("bq", [D], f32, kind="ExternalInput")      # bq/8
    d_bk = nc.dram_tensor("bk", [D], f32, kind="ExternalInput")
    d_bv = nc.dram_tensor("bv", [D], f32, kind="ExternalInput")
    d_bo = nc.dram_tensor("bo", [D], f32, kind="ExternalInput")
    d_out = nc.dram_tensor("out", [S, D], f32, kind="ExternalOutput")
    d_rec = nc.dram_tensor("rec_dram", [H, S], f32)

    Exp = mybir.ActivationFunctionType.Exp

    with tile.TileContext(nc) as tc, \
         tc.tile_pool(name="persist", bufs=1) as persist:

        qT = persist.tile([P, NET, S], fp16)             # [e%128, et, s]
        kT = persist.tile([P, NET, S], fp16)
        v_aug = persist.tile([P, NST, H, DK + 1], fp16)  # [j%128, jt, h, d|1]
        outT = persist.tile([P, NET, S], fp16)           # [hd%128, et, s] unnorm
        denom = persist.tile([P, NSH, 64], f32)
        bq_sb = persist.tile([P, NET], f32)
        bk_sb = persist.tile([P, NET], f32)
        bv_bc = persist.tile([P, D], f32)
        wo_sb = persist.tile([P, NET, D], fp16)
        bo_bc = persist.tile([P, D], f32)
        outTn = persist.tile([P, NET, S], fp16)

        nc.sync.dma_start(out=bq_sb, in_=d_bq.ap().rearrange("(cc p) -> p cc", p=P))
        nc.sync.dma_start(out=bk_sb, in_=d_bk.ap().rearrange("(cc p) -> p cc", p=P))
        nc.sync.dma_start(
            out=bv_bc,
            in_=bass.AP(tensor=d_bv.ap().tensor, offset=0, ap=[[0, P], [1, D]]))
        nc.vector.memset(v_aug[:, :, :, DK:DK + 1], 1.0)

        with tc.tile_pool(name="maskp", bufs=1) as maskp:
          maskT = maskp.tile([P, NST, S], fp16)
          msk_ap = d_mskT.ap().rearrange("(jt p) s -> p jt s", p=P)

          # ---------------- projections (q, k, v) ----------------
          with tc.tile_pool(name="projx", bufs=2) as projx, \
               tc.tile_pool(name="projw", bufs=2) as projw, \
               tc.tile_pool(name="projps", bufs=4, space="PSUM") as projps:
            # mask is consumed jt-by-jt once attention starts (~45us in);
            # keep it off the x/w critical path.
            mask_sched = {0: [], 1: range(0, 6), 2: range(6, NST)}
            for which, (d_x, d_w) in enumerate(
                    [(d_xq, d_wq), (d_xk, d_wk), (d_xv, d_wv)]):
                w_sb = projw.tile([P, NET, D], fp16, tag="w", name="w_sb")
                nc.sync.dma_start(
                    out=w_sb, in_=d_w.ap().rearrange("(cc p) e -> p cc e", p=P))
                x_sb = projx.tile([P, NET, S], fp16, tag="x", name="x_sb")
                x_ap = d_x.ap().rearrange("(cc p) s -> p cc s", p=P)
                for cc in range(NET):
                    nc.sync.dma_start(out=x_sb[:, cc, :], in_=x_ap[:, cc, :])
                if which == 2:
                    nc.sync.dma_start(
                        out=wo_sb,
                        in_=d_wo.ap().rearrange("(cc p) e -> p cc e", p=P))
                    nc.sync.dma_start(
                        out=bo_bc,
                        in_=bass.AP(tensor=d_bo.ap().tensor, offset=0,
                                    ap=[[0, P], [1, D]]))
                for jt in mask_sched[which]:
                    nc.sync.dma_start(out=maskT[:, jt, :], in_=msk_ap[:, jt, :])

                if which == 2:  # v -> natural layout [s, e] into v_aug
                    for st in range(NST):
                        ps_t = projps.tile([P, SCW], f32, tag="ps",
                                           name="ps_t")
                        for cc in range(NET):
                            nc.tensor.matmul(
                                ps_t,
                                x_sb[:, cc, st * P:(st + 1) * P],
                                w_sb[:, cc, :],
                                start=(cc == 0), stop=(cc == NET - 1))
                        nc.vector.tensor_add(
                            v_aug[:, st, :, 0:DK],
                            ps_t.rearrange("p (h d) -> p h d", h=H),
                            bv_bc.rearrange("p (h d) -> p h d", h=H))
                else:  # q, k -> transposed layout [e, s]
                    dst = qT if which == 0 else kT
                    bias = bq_sb if which == 0 else bk_sb
                    for et in range(NET):
                        for sc in range(NSC):
                            ps_t = projps.tile([P, SCW], f32, tag="ps",
                                               name="ps_t")
                            for cc in range(NET):
                                nc.tensor.matmul(
                                    ps_t,
                                    w_sb[:, cc, et * P:(et + 1) * P],
                                    x_sb[:, cc, sc * SCW:(sc + 1) * SCW],
                                    start=(cc == 0), stop=(cc == NET - 1))
                            nc.scalar.activation(
                                dst[:, et, sc * SCW:(sc + 1) * SCW], ps_t,
                                mybir.ActivationFunctionType.Identity,
                                bias=bias[:, et:et + 1])

          # ---------------- attention ----------------
          # sh outer: each s-block is normalized and projected while the
          # next one computes, so the tail stays short.
          with tc.tile_pool(name="attn", bufs=4) as attn, \
               tc.tile_pool(name="attnps", bufs=1, space="PSUM") as attnps:
            for sh in range(NSH):
                c0 = sh * SHW
                for et in range(NET):          # head pair (2et, 2et+1)
                    pvs = [attnps.tile([65, SCW], f32, tag="pv", bufs=4,
                                       name=f"pv{hh}") for hh in range(2)]
                    for jt in range(NST):
                        # both heads' scores in one 2-bank double-buffered
                        # tile; the two K=64 matmuls row-tile and run
                        # concurrently on the PE.
                        sc_ps = attnps.tile([P, 2, SHW], f32, tag="sc",
                                            bufs=2, name="sc_ps")
                        for hh in range(2):
                            nc.tensor.matmul(
                                sc_ps[:, hh, :],
                                kT[hh * DK:(hh + 1) * DK, et,
                                   jt * P:(jt + 1) * P],
                                qT[hh * DK:(hh + 1) * DK, et,
                                   c0:c0 + SHW],
                                start=True, stop=True)
                        ex = attn.tile([P, 2, SHW], fp16, tag="ex", bufs=3,
                                       name="ex")
                        nc.scalar.activation(ex, sc_ps, Exp)  # FD=1024
                        pb = attn.tile([P, 2, SHW], fp16, tag="pb", bufs=6,
                                       name="pb")
                        msk = maskT[:, jt, c0:c0 + SHW]
                        msk2 = bass.AP(tensor=msk.tensor, offset=msk.offset,
                                       ap=[msk.ap[0], [0, 2]] + msk.ap[1:])
                        nc.vector.tensor_mul(pb, ex, msk2)
                        for hh in range(2):
                            nc.tensor.matmul(
                                pvs[hh],
                                v_aug[:, jt, 2 * et + hh, :],
                                pb[:, hh, :],
                                start=(jt == 0), stop=(jt == NST - 1))
                    for hh in range(2):
                        ro = hh * DK
                        nc.vector.tensor_copy(
                            outT[ro:ro + DK, et, c0:c0 + SHW],
                            pvs[hh][0:DK, :])
                        dst_t = attn.tile([65, SCW], f32, tag="dst",
                                          bufs=2, name="dst_t")
                        nc.vector.tensor_copy(dst_t[64:65, :],
                                              pvs[hh][64:65, :])
                        pbase = et * 32 + hh * 8
                        nc.gpsimd.dma_start(
                            out=denom[pbase:pbase + 8, sh, :],
                            in_=dst_t[64:65, :])
                    # pair done for this block: normalize now
                    rec = attn.tile([16, 64], f32, tag="rec", bufs=2,
                                    name="rec")
                    nc.vector.reciprocal(
                        rec, denom[et * 32:et * 32 + 16, sh, :])
                    nc.sync.dma_start(
                        out=d_rec.ap()[2 * et:2 * et + 2, c0:c0 + SHW],
                        in_=rec)
                    rb = attn.tile([P, SHW], f32, tag="rb", bufs=2,
                                   name="rb")
                    nc.gpsimd.dma_start(
                        out=rb[0:64, :],
                        in_=bass.AP(tensor=d_rec.ap().tensor,
                                    offset=(2 * et) * S + c0,
                                    ap=[[0, 64], [1, SHW]]))
                    nc.gpsimd.dma_start(
                        out=rb[64:128, :],
                        in_=bass.AP(tensor=d_rec.ap().tensor,
                                    offset=(2 * et + 1) * S + c0,
                                    ap=[[0, 64], [1, SHW]]))
                    nc.vector.tensor_mul(outTn[:, et, c0:c0 + SHW],
                                         outT[:, et, c0:c0 + SHW], rb)
                # final projection for this s-block (PSUM slots shared w/ pv)
                for st in range(sh * NST // NSH, (sh + 1) * NST // NSH):
                    ps_f = attnps.tile([P, D], f32, tag="pv", bufs=4,
                                       name="ps_f")
                    for cc in range(NET):
                        nc.tensor.matmul(
                            ps_f,
                            outTn[:, cc, st * P:(st + 1) * P],
                            wo_sb[:, cc, :],
                            start=(cc == 0), stop=(cc == NET - 1))
                    o_sb = attn.tile([P, D], f32, tag="os", bufs=2,
                                     name="o_sb")
                    nc.vector.tensor_add(o_sb, ps_f, bo_bc)
                    nc.sync.dma_start(
                        out=d_out.ap()[st * P:(st + 1) * P, :], in_=o_sb)

    nc.compile()
    return nc


def _get_nc():
    if "nc" not in _CACHE:
        _CACHE["nc"] = _build()
    return _CACHE["nc"]


def _preprocess(Q, K, V, mask, Wq, bq, Wk, bk, Wv, bv, Wo, bo):
    """Host-side sharding + layout marshalling (per-core input dicts)."""
    mT = np.ascontiguousarray(np.asarray(mask)[0, 0].T).astype(np.float16)
    wq_h = np.ascontiguousarray(np.asarray(Wq).T / 8.0).astype(np.float16)
    wk_h = np.ascontiguousarray(np.asarray(Wk).T).astype(np.float16)
    wv_h = np.ascontiguousarray(np.asarray(Wv).T).astype(np.float16)
    wo_h = np.ascontiguousarray(np.asarray(Wo).T).astype(np.float16)
    bq_h = np.asarray(bq, dtype=np.float32) / 8.0
    bk_h = np.asarray(bk, dtype=np.float32)
    bv_h = np.asarray(bv, dtype=np.float32)
    bo_h = np.asarray(bo, dtype=np.float32)
    Q, K, V = np.asarray(Q), np.asarray(K), np.asarray(V)
    in_maps = []
    for b in range(B):
        in_maps.append({
            "xq": np.ascontiguousarray(Q[b].T).astype(np.float16),
            "xk": np.ascontiguousarray(K[b].T).astype(np.float16),
            "xv": np.ascontiguousarray(V[b].T).astype(np.float16),
            "mskT": mT,
            "wq": wq_h, "wk": wk_h, "wv": wv_h, "wo": wo_h,
            "bq": bq_h, "bk": bk_h, "bv": bv_h, "bo": bo_h,
        })
    return in_maps


def run(inputs: dict, trace: bool = False):
    nc = _get_nc()
    in_maps = _preprocess(**inputs)
    res = run_bass_kernel_spmd(nc, in_maps, core_ids=list(range(B)), trace=trace)
    outp = np.stack([res.results[b]["out"] for b in range(B)], axis=0)
    return outp.astype(np.float32), res


def kernel(**inputs) -> np.ndarray:
    outp, _ = run(inputs, trace=False)
    return outp


# revision 13
# speedup vs baseline: 1.2398x; 1.0398x over previous
"""Multi-head attention (B=8, S=2048, D=512, H=8) on 8 Trainium2 NeuronCores.

Strategy: pure data parallelism — one batch element per core, no collectives.

Per-core device pipeline (all matmuls fp16 with fp32 PSUM accumulation):
  1. Projections: qT/kT in transposed layout [e, s] (attention contracts
     dk on partitions), v in natural [s, e] layout augmented with a ones
     column per head (the PV matmul then also produces softmax denominators).
     Inputs arrive pre-transposed from host as X^T [c, s] fp16.  Projection
     issue is interleaved with attention so the ScalarE exp stream starts
     as soon as qT/kT for the first head pair exist (~20us in) instead of
     after the whole projection phase.
  2. Attention per (head-pair, s-block of 512): the two heads of an e-tile
     live on partitions 0-63 / 64-127, so their K=64 score matmuls are
     issued back-to-back as PE row-tiles (tile_position (0,0)/(64,0)) and
     run concurrently into one [128, 2, 512] PSUM tile (2 banks,
     double-buffered).  One FD=1024 ScalarE exp covers the pair
     (PSUM->SBUF fp16), one DVE multiply applies the 0/1 mask to both
     heads via a zero-stride broadcast AP, then two PV matmuls with [V|1]
     stationary accumulate outT rows + denominators in PSUM.  The
     pipeline is ScalarE-bound (33.5M exps/core at 1 elem/lane/cycle);
     PE and DVE have slack.
  3. Denominator rows staged to SBUF, DVE reciprocal, partition-broadcast
     via a DRAM bounce, in-place normalize, final projection with Wo.T
     (issued after the next s-block's first pair so the exp stream never
     waits on it), bias, DMA out.

Softmax note: reference softmax(where(mask==0, -1e30, s)) == exp(s)*mask
normalized — scores are O(1) so no max-subtraction is needed, and the 0/1
mask is exact in fp16. Scale 1/sqrt(dk)=1/8 is folded into Wq/bq on host.
"""
import numpy as np

import concourse.bacc as bacc
import concourse.bass as bass
import concourse.mybir as mybir
import concourse.tile as tile
from concourse.bass_utils import run_bass_kernel_spmd

B, S, D, H, DK = 8, 2048, 512, 8, 64
P = 128            # partition tile
NET = D // P       # 4 e-tiles (contraction chunks / head pairs)
NST = S // P       # 16 s-tiles / j-tiles
SCW = 512          # matmul moving free dim
NSC = S // SCW     # 4
SHW = 512          # attention s-block width
NSH = S // SHW     # 4

f32 = mybir.dt.float32
fp16 = mybir.dt.float16

_CACHE: dict = {}


def _bcast(ap, n):
    """Repeat a [P, w] AP n times along a new middle dim (stride 0)."""
    return bass.AP(tensor=ap.tensor, offset=ap.offset,
                   ap=[ap.ap[0], [0, n]] + ap.ap[1:])


def _build():
    nc = bacc.Bacc("TRN2", target_bir_lowering=False, debug=False)

    d_xq = nc.dram_tensor("xq", [D, S], fp16, kind="ExternalInput")
    d_xk = nc.dram_tensor("xk", [D, S], fp16, kind="ExternalInput")
    d_xv = nc.dram_tensor("xv", [D, S], fp16, kind="ExternalInput")
    # mask chunked by s-block on host: [NSH, S(j), SHW]
    d_mskT = nc.dram_tensor("mskT", [NSH, S, SHW], fp16, kind="ExternalInput")
    d_wq = nc.dram_tensor("wq", [D, D], fp16, kind="ExternalInput")  # Wq.T/8
    d_wk = nc.dram_tensor("wk", [D, D], fp16, kind="ExternalInput")  # Wk.T
    d_wv = nc.dram_tensor("wv", [D, D], fp16, kind="ExternalInput")  # Wv.T
    d_wo = nc.dram_tensor("wo", [D, D], fp16, kind="ExternalInput")  # Wo.T
    d_bq = nc.dram_tensor("bq", [D], f32, kind="ExternalInput")      # bq/8
    d_bk = nc.dram_tensor("bk", [D], f32, kind="ExternalInput")
    d_bv = nc.dram_tensor("bv", [D], f32, kind="ExternalInput")
    d_bo = nc.dram_tensor("bo", [D], f32, kind="ExternalInput")
    d_out = nc.dram_tensor("out", [S, D], f32, kind="ExternalOutput")
    d_rec = nc.dram_tensor("rec_dram", [H, S], f32)

    Exp = mybir.ActivationFunctionType.Exp

    with tile.TileContext(nc) as tc, \
         tc.tile_pool(name="persist", bufs=1) as persist, \
         tc.tile_pool(name="maskp", bufs=2) as maskp, \
         tc.tile_pool(name="projx", bufs=3) as projx, \
         tc.tile_pool(name="projw", bufs=3) as projw, \
         tc.tile_pool(name="attn", bufs=4) as attn, \
         tc.tile_pool(name="ps", bufs=1, space="PSUM") as psp:

        qT = persist.tile([P, NET, S], fp16)             # [e%128, et, s]
        kT = persist.tile([P, NET, S], fp16)
        v_aug = persist.tile([P, NST, H, DK + 1], fp16)  # [j%128, jt, h, d|1]
        outT = persist.tile([P, NET, S], fp16)           # [hd%128, et, s]
        denom = persist.tile([P, NSH, 64], f32)
        bq_sb = persist.tile([P, NET], f32)
        bk_sb = persist.tile([P, NET], f32)
        bv_bc = persist.tile([P, D], f32)
        wo_sb = persist.tile([P, NET, D], fp16)
        bo_bc = persist.tile([P, D], f32)
        warm = persist.tile([P, 2], f32)

        # small constants / biases first, then the exp table preload runs
        # while the big input DMAs stream.
        nc.sync.dma_start(out=bq_sb, in_=d_bq.ap().rearrange("(cc p) -> p cc", p=P))
        nc.sync.dma_start(out=bk_sb, in_=d_bk.ap().rearrange("(cc p) -> p cc", p=P))
        nc.sync.dma_start(
            out=bv_bc,
            in_=bass.AP(tensor=d_bv.ap().tensor, offset=0, ap=[[0, P], [1, D]]))
        nc.vector.memset(v_aug[:, :, :, DK:DK + 1], 1.0)
        nc.scalar.activation(warm[:, 0:1], bq_sb[:, 0:1], Exp)  # table preload

        # ---------------- input DMAs (issue order = criticality) --------
        x_sbs, w_sbs = [], []
        for which, (d_x, d_w) in enumerate(
                [(d_xq, d_wq), (d_xk, d_wk), (d_xv, d_wv)]):
            w_sb = projw.tile([P, NET, D], fp16, tag="w", name=f"w{which}")
            nc.sync.dma_start(
                out=w_sb, in_=d_w.ap().rearrange("(cc p) e -> p cc e", p=P))
            x_sb = projx.tile([P, NET, S], fp16, tag="x", name=f"x{which}")
            x_ap = d_x.ap().rearrange("(cc p) s -> p cc s", p=P)
            for cc in range(NET):
                nc.sync.dma_start(out=x_sb[:, cc, :], in_=x_ap[:, cc, :])
            x_sbs.append(x_sb)
            w_sbs.append(w_sb)
        nc.sync.dma_start(
            out=wo_sb, in_=d_wo.ap().rearrange("(cc p) e -> p cc e", p=P))
        nc.sync.dma_start(
            out=bo_bc,
            in_=bass.AP(tensor=d_bo.ap().tensor, offset=0, ap=[[0, P], [1, D]]))

        def load_mask(sh):
            m = maskp.tile([P, NST, SHW], fp16, tag="m", name=f"m{sh}")
            nc.sync.dma_start(
                out=m, in_=d_mskT.ap()[sh].rearrange("(jt p) w -> p jt w", p=P))
            return m

        def proj_qk(which, et):
            dst = qT if which == 0 else kT
            bias = bq_sb if which == 0 else bk_sb
            for sc in range(NSC):
                ps_t = psp.tile([P, SCW], f32, tag="pp", bufs=2, name="ps_t")
                for cc in range(NET):
                    nc.tensor.matmul(
                        ps_t,
                        w_sbs[which][:, cc, et * P:(et + 1) * P],
                        x_sbs[which][:, cc, sc * SCW:(sc + 1) * SCW],
                        start=(cc == 0), stop=(cc == NET - 1))
                nc.vector.tensor_scalar_add(
                    dst[:, et, sc * SCW:(sc + 1) * SCW], ps_t,
                    bias[:, et:et + 1])

        def proj_v(st_lo, st_hi):
            for st in range(st_lo, st_hi):
                ps_t = psp.tile([P, SCW], f32, tag="pp", bufs=2, name="ps_t")
                for cc in range(NET):
                    nc.tensor.matmul(
                        ps_t,
                        x_sbs[2][:, cc, st * P:(st + 1) * P],
                        w_sbs[2][:, cc, :],
                        start=(cc == 0), stop=(cc == NET - 1))
                nc.vector.tensor_add(
                    v_aug[:, st, :, 0:DK],
                    ps_t.rearrange("p (h d) -> p h d", h=H),
                    bv_bc.rearrange("p (h d) -> p h d", h=H))

        def attn_pair(sh, et, msk_sh):
            c0 = sh * SHW
            pvs = [psp.tile([65, SCW], f32, tag="pv", bufs=2,
                            name=f"pv{hh}") for hh in range(2)]
            for jt in range(NST):
                sc_ps = psp.tile([P, 2, SHW], f32, tag="sc", bufs=2,
                                 name="sc_ps")
                for hh in range(2):   # K=64 row-tiled pair, runs concurrent
                    nc.tensor.matmul(
                        sc_ps[:, hh, :],
                        kT[hh * DK:(hh + 1) * DK, et, jt * P:(jt + 1) * P],
                        qT[hh * DK:(hh + 1) * DK, et, c0:c0 + SHW],
                        start=True, stop=True)
                ex = attn.tile([P, 2, SHW], fp16, tag="ex", bufs=3, name="ex")
                nc.scalar.activation(ex, sc_ps, Exp)  # FD=1024
                pb = attn.tile([P, 2, SHW], fp16, tag="pb", bufs=6, name="pb")
                nc.vector.tensor_mul(pb, ex, _bcast(msk_sh[:, jt, :], 2))
                for hh in range(2):
                    nc.tensor.matmul(
                        pvs[hh], v_aug[:, jt, 2 * et + hh, :], pb[:, hh, :],
                        start=(jt == 0), stop=(jt == NST - 1))
            for hh in range(2):
                ro = hh * DK
                nc.vector.tensor_copy(
                    outT[ro:ro + DK, et, c0:c0 + SHW], pvs[hh][0:DK, :])
                dst_t = attn.tile([65, SCW], f32, tag="dst", bufs=2,
                                  name="dst_t")
                nc.vector.tensor_copy(dst_t[64:65, :], pvs[hh][64:65, :])
                pbase = et * 32 + hh * 8
                nc.gpsimd.dma_start(
                    out=denom[pbase:pbase + 8, sh, :], in_=dst_t[64:65, :])
            # pair done for this block: reciprocal + broadcast + normalize
            rec = attn.tile([16, 64], f32, tag="rec", bufs=2, name="rec")
            nc.vector.reciprocal(rec, denom[et * 32:et * 32 + 16, sh, :])
            nc.sync.dma_start(
                out=d_rec.ap()[2 * et:2 * et + 2, c0:c0 + SHW], in_=rec)
            rb = attn.tile([P, SHW], f32, tag="rb", bufs=2, name="rb")
            nc.gpsimd.dma_start(
                out=rb[0:64, :],
                in_=bass.AP(tensor=d_rec.ap().tensor,
                            offset=(2 * et) * S + c0, ap=[[0, 64], [1, SHW]]))
            nc.gpsimd.dma_start(
                out=rb[64:128, :],
                in_=bass.AP(tensor=d_rec.ap().tensor,
                            offset=(2 * et + 1) * S + c0,
                            ap=[[0, 64], [1, SHW]]))
            nc.vector.tensor_mul(outT[:, et, c0:c0 + SHW],
                                 outT[:, et, c0:c0 + SHW], rb)

        def final_proj(sh):
            for st in range(sh * NST // NSH, (sh + 1) * NST // NSH):
                ps_f = psp.tile([P, D], f32, tag="pp", bufs=2, name="ps_f")
                for cc in range(NET):
                    nc.tensor.matmul(
                        ps_f,
                        outT[:, cc, st * P:(st + 1) * P],
                        wo_sb[:, cc, :],
                        start=(cc == 0), stop=(cc == NET - 1))
                o_sb = attn.tile([P, D], f32, tag="os", bufs=2, name="o_sb")
                nc.vector.tensor_add(o_sb, ps_f, bo_bc)
                nc.sync.dma_start(
                    out=d_out.ap()[st * P:(st + 1) * P, :], in_=o_sb)

        # ---------------- issue schedule ----------------
        # attention for (sh0, pair0) as early as possible; remaining
        # projections ride the PE slack under the ScalarE-bound stream.
        proj_qk(0, 0)
        proj_qk(1, 0)
        masks = {0: load_mask(0)}
        proj_v(0, NST)
        attn_pair(0, 0, masks[0])
        for et in range(1, NET):
            proj_qk(0, et)
            proj_qk(1, et)
            attn_pair(0, et, masks[0])
            if et == 1:
                masks[1] = load_mask(1)
        for sh in range(1, NSH):
            if sh + 1 < NSH:
                masks[sh + 1] = load_mask(sh + 1)
            for et in range(NET):
                attn_pair(sh, et, masks[sh])
                if et == 0:
                    final_proj(sh - 1)
        final_proj(NSH - 1)

    nc.compile()
    return nc


def _get_nc():
    if "nc" not in _CACHE:
        _CACHE["nc"] = _build()
    return _CACHE["nc"]


def _preprocess(Q, K, V, mask, Wq, bq, Wk, bk, Wv, bv, Wo, bo):
    """Host-side sharding + layout marshalling (per-core input dicts)."""
    mT = np.ascontiguousarray(np.asarray(mask)[0, 0].T).astype(np.float16)
    # chunk columns by s-block: [NSH, S(j), SHW]
    mTc = np.ascontiguousarray(mT.reshape(S, NSH, SHW).transpose(1, 0, 2))
    wq_h = np.ascontiguousarray(np.asarray(Wq).T / 8.0).astype(np.float16)
    wk_h = np.ascontiguousarray(np.asarray(Wk).T).astype(np.float16)
    wv_h = np.ascontiguousarray(np.asarray(Wv).T).astype(np.float16)
    wo_h = np.ascontiguousarray(np.asarray(Wo).T).astype(np.float16)
    bq_h = np.asarray(bq, dtype=np.float32) / 8.0
    bk_h = np.asarray(bk, dtype=np.float32)
    bv_h = np.asarray(bv, dtype=np.float32)
    bo_h = np.asarray(bo, dtype=np.float32)
    Q, K, V = np.asarray(Q), np.asarray(K), np.asarray(V)
    in_maps = []
    for b in range(B):
        in_maps.append({
            "xq": np.ascontiguousarray(Q[b].T).astype(np.float16),
            "xk": np.ascontiguousarray(K[b].T).astype(np.float16),
            "xv": np.ascontiguousarray(V[b].T).astype(np.float16),
            "mskT": mTc,
            "wq": wq_h, "wk": wk_h, "wv": wv_h, "wo": wo_h,
            "bq": bq_h, "bk": bk_h, "bv": bv_h, "bo": bo_h,
        })
    return in_maps


def run(inputs: dict, trace: bool = False):
    nc = _get_nc()
    in_maps = _preprocess(**inputs)
    res = run_bass_kernel_spmd(nc, in_maps, core_ids=list(range(B)), trace=trace)
    outp = np.stack([res.results[b]["out"] for b in range(B)], axis=0)
    return outp.astype(np.float32), res


def kernel(**inputs) -> np.ndarray:
    outp, _ = run(inputs, trace=False)
    return outp
